# revision 25
# baseline (speedup 1.0000x reference)
"""Trainium2 Bass kernel: dense transformer block (LN1-attn-LN2-FFN, causal, 16 heads).

Sharding (8 NeuronCores, SPMD one graph):
  - core j: token-parallel for LN/FFN/residual: owns tokens [512l, 512(l+1))
    of batch g, where g, l = divmod(j, 4)
  - attention head-parallel with cyclic head-batch assignment: core j computes
    head pair {2m, 2m+1}, m = (j + 4b) % 8, for EACH batch b over the full
    2048-token sequence. Uniform causal loop structure on every core; all
    per-core variation (which heads / which tokens) lives in the input data.
  - comm: 8-core AllGather of LN1^T output (QKV sees all tokens), 8-core
    AllToAll of normalized attention^T (head-shard -> token-shard). The
    receive-side head permutation is folded into host-permuted wo rows.
  - matmuls bf16 (f32 accumulate); residual stream f32; softmax without
    max-subtraction (scores are O(1) for this problem scale).
  - LN gains/biases, 1/sqrt(dk), and bv are folded into weights host-side.
"""

import numpy as np
import ml_dtypes

import concourse.bass as bass
import concourse.tile as tile
from concourse import bacc, mybir
from concourse.bass_utils import run_bass_kernel_spmd

F32 = mybir.dt.float32
BF16 = mybir.dt.bfloat16
AF = mybir.ActivationFunctionType

D = 1024
DFF = 4096
B = 2
S = 2048
NCORES = 8
GRP = 4
TOK = 512        # tokens per core (FFN/LN shard)
EPS = 1e-5

AG_IN = D * TOK          # bf16 elems contributed to xln AllGather
A2A_N = NCORES * 128 * TOK   # total elems in the 8-way AllToAll


def build_nc():
    nc = bacc.Bacc("TRN2", target_bir_lowering=False, debug=False,
                   num_devices=NCORES)

    x_own = nc.dram_tensor("x_own", [TOK, D], F32, kind="ExternalInput").ap()
    wq = nc.dram_tensor("wq", [D, B, 128], BF16, kind="ExternalInput").ap()
    wk = nc.dram_tensor("wk", [D, B, 128], BF16, kind="ExternalInput").ap()
    wv = nc.dram_tensor("wv", [D, B, 128], BF16, kind="ExternalInput").ap()
    bq = nc.dram_tensor("bq", [B, 128], F32, kind="ExternalInput").ap()
    bk = nc.dram_tensor("bk", [B, 128], F32, kind="ExternalInput").ap()
    wo = nc.dram_tensor("wo", [D, D], BF16, kind="ExternalInput").ap()
    bo = nc.dram_tensor("bo", [D], F32, kind="ExternalInput").ap()
    w1 = nc.dram_tensor("w1", [D, DFF], BF16, kind="ExternalInput").ap()
    b1 = nc.dram_tensor("b1", [DFF], F32, kind="ExternalInput").ap()
    w2 = nc.dram_tensor("w2", [DFF, D], BF16, kind="ExternalInput").ap()
    b2 = nc.dram_tensor("b2", [D], F32, kind="ExternalInput").ap()
    id128 = nc.dram_tensor("id128", [128, 128], BF16, kind="ExternalInput").ap()
    masks = nc.dram_tensor("masks", [128, 4, 512], BF16,
                           kind="ExternalInput").ap()
    out = nc.dram_tensor("out", [TOK, D], F32, kind="ExternalOutput").ap()
    dbg_xg = nc.dram_tensor("dbg_xg", [128, 8, 4, 8, 128], BF16,
                            kind="ExternalOutput").ap()
    dbg_qT = nc.dram_tensor("dbg_qT", [128, 4, B, 512], BF16,
                            kind="ExternalOutput").ap()
    dbg_kT = nc.dram_tensor("dbg_kT", [128, 4, B, 512], BF16,
                            kind="ExternalOutput").ap()
    dbg_at = nc.dram_tensor("dbg_at", [128, B, S], BF16,
                            kind="ExternalOutput").ap()

    rg = [list(range(NCORES))]

    with tile.TileContext(nc) as tc:
        with (
            tc.tile_pool(name="persist", bufs=1) as pp,
            tc.tile_pool(name="stage", bufs=2) as stg,
            tc.tile_pool(name="stats", bufs=4) as stp,
            tc.tile_pool(name="ptp", bufs=6) as ptp,
            tc.tile_pool(name="rp", bufs=2) as rp,
            tc.tile_pool(name="psA", bufs=2, space="PSUM") as psA,
            tc.tile_pool(name="psS", bufs=2, space="PSUM") as psS,
            tc.tile_pool(name="psO", bufs=1, space="PSUM") as psO,
            tc.tile_pool(name="dram", bufs=1, space="DRAM") as dp,
        ):
            # ---- constants ----
            id_sb = pp.tile([128, 128], BF16, name="id_sb")
            nc.sync.dma_start(out=id_sb, in_=id128)
            mask_sb = pp.tile([128, 4, 512], BF16, name="mask_sb")
            nc.sync.dma_start(out=mask_sb, in_=masks)
            eps_sb = pp.tile([128, 1], F32, name="eps_sb")
            nc.vector.memset(eps_sb, EPS)
            bo_sb = pp.tile([128, 8], F32, name="bo_sb")
            nc.sync.dma_start(out=bo_sb, in_=bo.rearrange("(k p) -> p k", p=128))
            b1_sb = pp.tile([128, 32], F32, name="b1_sb")
            nc.sync.dma_start(out=b1_sb, in_=b1.rearrange("(k p) -> p k", p=128))
            b2_sb = pp.tile([128, 8], F32, name="b2_sb")
            nc.sync.dma_start(out=b2_sb, in_=b2.rearrange("(k p) -> p k", p=128))

            x_tok = pp.tile([128, 4, D], F32, name="x_tok")
            _xv = x_own.rearrange("(t p) d -> p t d", p=128)

            lnT = pp.tile([128, 8, TOK], BF16, name="lnT")

            def layernorm_block(src_tile, t):
                xin = src_tile[:, t, :]
                xg = xin.rearrange("p (g d) -> p g d", g=2)
                stats = stp.tile([128, 2, 6], F32, name="stats")
                for gsub in range(2):
                    nc.vector.bn_stats(out=stats[:, gsub, :],
                                       in_=xg[:, gsub, :])
                mv = stp.tile([128, 2], F32, name="mv")
                nc.vector.bn_aggr(out=mv, in_=stats)
                rstd = stp.tile([128, 1], F32, name="rstd")
                nc.scalar.activation(out=rstd, in_=mv[:, 1:2], func=AF.Sqrt,
                                     bias=eps_sb, scale=1.0)
                nc.vector.reciprocal(out=rstd, in_=rstd)
                xln = stg.tile([128, D], BF16, name="xln")
                nc.vector.tensor_scalar(out=xln, in0=xin, scalar1=mv[:, 0:1],
                                        scalar2=rstd,
                                        op0=mybir.AluOpType.subtract,
                                        op1=mybir.AluOpType.mult)
                for fb in range(8):
                    pt = psA.tile([128, 128], BF16, name="acc")
                    nc.tensor.transpose(pt, xln[:, fb * 128:(fb + 1) * 128],
                                        id_sb)
                    nc.vector.tensor_copy(
                        out=lnT[:, fb, t * 128:(t + 1) * 128], in_=pt)

            def layernorm_into_lnT(src_tile):
                for t in range(4):
                    layernorm_block(src_tile, t)

            # ---- chunked AllGather of xln^T, pipelined with LN1 + QKV ----
            CH = 128 * 8 * 128  # per-core elems per chunk (8 fb x 128 tok)
            ag_in = dp.tile([4, CH], BF16, name="ag_in")
            ag_outs = [dp.tile([NCORES * CH], BF16, name=f"ag_out{c}",
                               addr_space="Shared") for c in range(4)]
            a2a_in = dp.tile([A2A_N], BF16, name="a2a_in")
            a2a_out = dp.tile([A2A_N], BF16, name="a2a_out")

            with tc.tile_pool(name="attnp", bufs=1) as ap_:
                wq_sb = ap_.tile([128, 8, B, 128], BF16, name="wq_sb")
                nc.sync.dma_start(out=wq_sb,
                                  in_=wq.rearrange("(k p) b m -> p k b m", p=128))
                wk_sb = ap_.tile([128, 8, B, 128], BF16, name="wk_sb")
                nc.sync.dma_start(out=wk_sb,
                                  in_=wk.rearrange("(k p) b m -> p k b m", p=128))
                wv_sb = ap_.tile([128, 8, B, 128], BF16, name="wv_sb")
                nc.sync.dma_start(out=wv_sb,
                                  in_=wv.rearrange("(k p) b m -> p k b m", p=128))
                bq_sb = ap_.tile([128, B], F32, name="bq_sb")
                nc.sync.dma_start(out=bq_sb, in_=bq.rearrange("b p -> p b"))
                bk_sb = ap_.tile([128, B], F32, name="bk_sb")
                nc.sync.dma_start(out=bk_sb, in_=bk.rearrange("b p -> p b"))
                wo_sb = pp.tile([128, 8, 1024], BF16, name="wo_sb")
                nc.sync.dma_start(
                    out=wo_sb, in_=wo.rearrange("(k p) m -> p k m", p=128))

                for t in range(4):
                    nc.sync.dma_start(out=x_tok[:, t, :], in_=_xv[:, t, :])

                # chunk wire format is partition-major [p, a, t] so the SBUF
                # source AP keeps its partition dim first
                _agi = ag_in.rearrange("c (p a t) -> c p a t", a=8, p=128)
                for t in range(4):
                    layernorm_block(x_tok, t)
                    nc.sync.dma_start(
                        out=_agi[t],
                        in_=lnT[:, :, t * 128:(t + 1) * 128])
                    nc.gpsimd.collective_compute(
                        "AllGather", mybir.AluOpType.bypass, replica_groups=rg,
                        ins=[ag_in[t].opt()], outs=[ag_outs[t].opt()])

                # xg chunk-major: [128, kc, c, r, 128]; token (b,s) lives at
                # c=(s%512)//128, r=4b+s//512, i=s%128
                xg_sb = ap_.tile([128, 8, 4, 8, 128], BF16, name="xg_sb")
                ag4 = [g.rearrange("(r p a t) -> r p a t", r=NCORES, a=8,
                                   p=128) for g in ag_outs]

                # qT/kT: [128, c, b, l*128+i]; vtok slot = b*16 + 4l + c
                qT = ap_.tile([128, 4, B, 512], BF16, name="qT")
                kT = ap_.tile([128, 4, B, 512], BF16, name="kT")
                vtok = ap_.tile([128, 32, 2, 128], BF16, name="vtok")
                nc.vector.memset(vtok[:, :, :, 64:128], 1.0)

                for c in range(4):
                    for kc in range(8):
                        [nc.sync, nc.scalar][kc % 2].dma_start(
                            out=xg_sb[:, kc, c, :, :],
                            in_=ag4[c][:, :, kc, :].rearrange("r p t -> p r t"))
                    for b in range(B):
                        for dst, wsb, bsb in ((qT, wq_sb, bq_sb),
                                              (kT, wk_sb, bk_sb)):
                            acc = psA.tile([128, 512], F32, name="acc")
                            for kc in range(8):
                                nc.tensor.matmul(
                                    acc, lhsT=wsb[:, kc, b, :],
                                    rhs=xg_sb[:, kc, c, 4 * b:4 * b + 4, :],
                                    start=(kc == 0), stop=(kc == 7))
                            nc.vector.tensor_scalar_add(
                                out=dst[:, c, b, :], in0=acc,
                                scalar1=bsb[:, b:b + 1])
                        for l in range(4):
                            acc = psA.tile([128, 128], F32, name="acc")
                            for kc in range(8):
                                nc.tensor.matmul(
                                    acc, lhsT=xg_sb[:, kc, c, 4 * b + l, :],
                                    rhs=wv_sb[:, kc, b, :],
                                    start=(kc == 0), stop=(kc == 7))
                            nc.vector.tensor_copy(
                                out=vtok[:, b * 16 + 4 * l + c, :, 0:64],
                                in_=acc.rearrange("p (h c) -> p h c", h=2))

                # ---- causal attention: 2 heads x 2 batches, no max-sub.
                # q blocks of 512, k blocks of 128; the two heads' score
                # matmuls are row-packed (K=64 at PE rows 0/64) and run
                # concurrently in the PE array.
                nc.sync.dma_start(out=dbg_xg, in_=xg_sb)
                nc.sync.dma_start(out=dbg_qT, in_=qT)
                nc.sync.dma_start(out=dbg_kT, in_=kT)
                attnT = ap_.tile([128, B, S], BF16, name="attnT")
                _a2ai = a2a_in.rearrange("(s p t) -> s p t", s=8, p=128)
                for b in range(B):
                    for q4 in range(4):
                        O = [psO.tile([128, 512], F32, name=f"O{hh}")
                             for hh in range(2)]
                        nkt = 4 * q4 + 4

                        def emit_sc(kt, b=b, q4=q4):
                            # k block kt: chunk c=kt%4, cols 128*(kt//4)
                            # q block q4: cols 128*q4 of every chunk; the
                            # (c, i) order makes O's columns s-ascending
                            kl = 128 * (kt // 4)
                            sc = psS.tile([128, 2, 512], F32, name="sc")
                            for hh in range(2):
                                hp = hh * 64
                                nc.tensor.matmul(
                                    sc[:, hh, :],
                                    lhsT=kT[hp:hp + 64, kt % 4, b,
                                            kl:kl + 128],
                                    rhs=qT[hp:hp + 64, :, b,
                                           128 * q4:128 * q4 + 128],
                                    start=True, stop=True)
                            return sc

                        # software pipeline: scores for kt+1 are queued on
                        # the PE before the exp-dependent AV of kt, so the
                        # PE never idles behind the ACT engine
                        sc_cur = emit_sc(0)
                        for kt in range(nkt):
                            pt_ = ptp.tile([128, 2, 512], BF16, name="pt_")
                            nc.scalar.activation(out=pt_, in_=sc_cur,
                                                 func=AF.Exp)
                            if kt + 1 < nkt:
                                sc_cur = emit_sc(kt + 1)
                            m = kt - 4 * q4
                            if m >= 0:
                                for hh in range(2):
                                    nc.vector.tensor_mul(
                                        out=pt_[:, hh, :], in0=pt_[:, hh, :],
                                        in1=mask_sb[:, m, :])
                            for hh in range(2):
                                nc.tensor.matmul(
                                    O[hh], lhsT=vtok[:, b * 16 + kt, hh, :],
                                    rhs=pt_[:, hh, :],
                                    start=(kt == 0), stop=(kt == nkt - 1))
                        for hh in range(2):
                            den = rp.tile([64, 512], F32, name="den")
                            nc.vector.tensor_copy(out=den, in_=O[hh][64:128, :])
                            rec = rp.tile([64, 512], F32, name="rec")
                            nc.vector.reciprocal_approx_fast(out=rec, in_=den)
                            nc.vector.tensor_mul(
                                out=attnT[hh * 64:hh * 64 + 64, b,
                                          q4 * 512:(q4 + 1) * 512],
                                in0=O[hh][0:64, :], in1=rec)
                        # shard 4b+q4 of the A2A is exactly this (b, q4)
                        # slice; ship it while attention continues
                        nc.sync.dma_start(
                            out=_a2ai[4 * b + q4],
                            in_=attnT[:, b, q4 * 512:(q4 + 1) * 512])

            nc.gpsimd.collective_compute(
                "AllToAll", mybir.AluOpType.bypass, replica_groups=rg,
                ins=[a2a_in.opt()], outs=[a2a_out.opt()])
            nc.sync.dma_start(out=dbg_at, in_=attnT)

            with (
                tc.tile_pool(name="postp", bufs=1) as pc,
                tc.tile_pool(name="w1p", bufs=4) as w1p,
                tc.tile_pool(name="w2p", bufs=4) as w2p,
            ):
                af_sb = pc.tile([128, 8, TOK], BF16, name="af_sb")
                _af3 = a2a_out.rearrange("(i p t) -> i p t", i=8, p=128)
                for i in range(8):
                    [nc.gpsimd, nc.scalar][i % 2].dma_start(
                        out=af_sb[:, i, :], in_=_af3[i])

                # ---- wo projection + residual into x_tok (in place) ----
                yT = pc.tile([128, 8, TOK], BF16, name="yT")
                for fb in range(8):
                    acc = psA.tile([128, 512], F32, name="acc")
                    for kc in range(8):
                        nc.tensor.matmul(
                            acc, lhsT=wo_sb[:, kc, fb * 128:(fb + 1) * 128],
                            rhs=af_sb[:, kc, :],
                            start=(kc == 0), stop=(kc == 7))
                    nc.vector.tensor_scalar_add(out=yT[:, fb, :], in0=acc,
                                                scalar1=bo_sb[:, fb:fb + 1])
                for fb in range(8):
                    for t in range(4):
                        pt = psA.tile([128, 128], BF16, name="acc")
                        nc.tensor.transpose(pt, yT[:, fb, t * 128:(t + 1) * 128],
                                            id_sb)
                        nc.vector.tensor_add(
                            out=x_tok[:, t, fb * 128:(fb + 1) * 128],
                            in0=x_tok[:, t, fb * 128:(fb + 1) * 128], in1=pt)

                # ---- LN2 -> lnT (reused), FFN ----
                layernorm_into_lnT(x_tok)

                h1T = pc.tile([128, 32, TOK], BF16, name="h1T")
                for hbk in range(32):
                    w1t = w1p.tile([128, 8, 128], BF16, name="w1t")
                    nc.sync.dma_start(
                        out=w1t,
                        in_=w1[:, hbk * 128:(hbk + 1) * 128]
                        .rearrange("(k p) m -> p k m", p=128))
                    acc = psA.tile([128, 512], F32, name="acc")
                    for kc in range(8):
                        nc.tensor.matmul(acc, lhsT=w1t[:, kc, :],
                                         rhs=lnT[:, kc, :],
                                         start=(kc == 0), stop=(kc == 7))
                    nc.scalar.activation(out=h1T[:, hbk, :], in_=acc,
                                         func=AF.Gelu,
                                         bias=b1_sb[:, hbk:hbk + 1], scale=1.0)

                for fb in range(8):
                    acc = psA.tile([128, 512], F32, name="acc")
                    for hg in range(4):
                        w2t = w2p.tile([128, 8, 128], BF16, name="w2t")
                        nc.sync.dma_start(
                            out=w2t,
                            in_=w2[hg * 1024:(hg + 1) * 1024,
                                   fb * 128:(fb + 1) * 128]
                            .rearrange("(k p) m -> p k m", p=128))
                        for kc in range(8):
                            nc.tensor.matmul(acc, lhsT=w2t[:, kc, :],
                                             rhs=h1T[:, hg * 8 + kc, :],
                                             start=(hg == 0 and kc == 0),
                                             stop=(hg == 3 and kc == 7))
                    y2T = stg.tile([128, TOK], BF16, name="y2T")
                    nc.vector.tensor_scalar_add(out=y2T, in0=acc,
                                                scalar1=b2_sb[:, fb:fb + 1])
                    for t in range(4):
                        pt = psA.tile([128, 128], BF16, name="acc")
                        nc.tensor.transpose(pt, y2T[:, t * 128:(t + 1) * 128],
                                            id_sb)
                        nc.vector.tensor_add(
                            out=x_tok[:, t, fb * 128:(fb + 1) * 128],
                            in0=x_tok[:, t, fb * 128:(fb + 1) * 128], in1=pt)

                nc.sync.dma_start(out=out.rearrange("(t p) d -> p t d", p=128),
                                  in_=x_tok)
    nc.compile()
    return nc


_NC_CACHE = {}


def _get_nc():
    if "nc" not in _NC_CACHE:
        _NC_CACHE["nc"] = build_nc()
    return _NC_CACHE["nc"]


def _prep_in_maps(x, ln1_g, ln1_b, wq, bq, wk, bk, wv, bv, wo, bo,
                  ln2_g, ln2_b, w1, b1, w2, b2):
    bf16 = ml_dtypes.bfloat16
    f32 = np.float32
    x = np.asarray(x, f32)
    DK = 64
    sc = 1.0 / np.sqrt(DK)
    ln1_g = np.asarray(ln1_g, f32)
    ln1_b = np.asarray(ln1_b, f32)
    ln2_g = np.asarray(ln2_g, f32)
    ln2_b = np.asarray(ln2_b, f32)
    wq = np.asarray(wq, f32)
    wk = np.asarray(wk, f32)
    wv = np.asarray(wv, f32)
    wo_np = np.asarray(wo, f32)
    w1 = np.asarray(w1, f32)
    w2 = np.asarray(w2, f32)

    wq_f = (ln1_g[:, None] * wq * sc).astype(bf16)
    bq_f = ((ln1_b @ wq + np.asarray(bq, f32)) * sc).astype(f32)
    wk_f = (ln1_g[:, None] * wk).astype(bf16)
    bk_f = (ln1_b @ wk + np.asarray(bk, f32)).astype(f32)
    wv_f = (ln1_g[:, None] * wv).astype(bf16)
    bv_f = (ln1_b @ wv + np.asarray(bv, f32)).astype(f32)
    bo_f = (np.asarray(bo, f32) + bv_f @ wo_np).astype(f32)
    wo_f = wo_np.astype(bf16)
    w1_f = (ln2_g[:, None] * w1).astype(bf16)
    b1_f = (ln2_b @ w1 + np.asarray(b1, f32)).astype(f32)
    w2_f = w2.astype(bf16)
    b2_f = np.asarray(b2, f32)

    # masks[i, m, j] = 1 iff q position j (within a 512 block) >= k position
    # i + 128*m (k block m of the diagonal 512-token region)
    ii = np.arange(128)[:, None, None]
    mm_ = np.arange(4)[None, :, None]
    jj = np.arange(512)[None, None, :]
    masks_np = (jj >= ii + 128 * mm_).astype(f32).astype(bf16)
    id128 = np.eye(128, dtype=f32).astype(bf16)

    in_maps = []
    for core in range(NCORES):
        g, l = divmod(core, GRP)

        def hsel(b, j=core):
            m = (j + 4 * b) % 8
            return slice(m * 128, (m + 1) * 128)

        wo_perm = np.concatenate(
            [wo_f[((i + 4 * g) % 8) * 128:((i + 4 * g) % 8) * 128 + 128, :]
             for i in range(8)], axis=0)
        in_maps.append({
            "x_own": np.ascontiguousarray(x[g, l * TOK:(l + 1) * TOK, :]),
            "wq": np.ascontiguousarray(
                np.stack([wq_f[:, hsel(b)] for b in range(B)], axis=1)),
            "wk": np.ascontiguousarray(
                np.stack([wk_f[:, hsel(b)] for b in range(B)], axis=1)),
            "wv": np.ascontiguousarray(
                np.stack([wv_f[:, hsel(b)] for b in range(B)], axis=1)),
            "bq": np.ascontiguousarray(
                np.stack([bq_f[hsel(b)] for b in range(B)])),
            "bk": np.ascontiguousarray(
                np.stack([bk_f[hsel(b)] for b in range(B)])),
            "wo": np.ascontiguousarray(wo_perm), "bo": bo_f,
            "w1": w1_f, "b1": b1_f, "w2": w2_f, "b2": b2_f,
            "id128": id128, "masks": masks_np,
        })
    return in_maps


def kernel(**inputs):
    nc = _get_nc()
    in_maps = _prep_in_maps(**inputs)
    res = run_bass_kernel_spmd(nc, in_maps, core_ids=list(range(NCORES)))
    full = np.empty((B, S, D), np.float32)
    for core in range(NCORES):
        g, l = divmod(core, GRP)
        full[g, l * TOK:(l + 1) * TOK, :] = res.results[core]["out"]
    return full

